# revision 41
# baseline (speedup 1.0000x reference)
# Trainium2 Bass kernel for a BitLinear transformer block (attention + SwiGLU FFN).
#
# Sharding across 8 NeuronCores:
#   - rmsnorm1 + qkv: sequence-parallel rmsnorm (each core norms its 512
#     tokens), then a 1MB AllGather of xhat so core c can compute q/k/v for
#     its 2 global heads {2c, 2c+1} over ALL tokens (head-parallel qkv).
#   - Attention: head-parallel flash-style causal attention, both batches.
#   - AllToAll re-shards attention outputs from head-parallel to
#     token-parallel; out_proj / residual / rmsnorm2 / FFN run
#     sequence-parallel (512 tokens per core) with the full ternary weights.
#   - Weights are quantized/transposed to bf16 ternary on the host (the
#     per-matrix quant scales are folded into PSUM eviction scale factors on
#     device); each core streams them from its own HBM — no weight
#     collectives.
#
# Numerics: ternary {-1,0,1} weights are exact in bf16; matmuls run in bf16
# with fp32 PSUM accumulation; the residual stream stays fp32 end-to-end;
# softmax runs in fp32 without max-subtraction (logits are O(1) here);
# rsqrt for both rmsnorms is computed as exp(-0.5*log(v)) so the scalar
# engine keeps one activation table (natural_log_exp) through attention.

import numpy as np

DR_QKV = True  # DoubleRow fp8 matmuls in qkv
DR_FFN = True   # DoubleRow fp8 matmuls in FFN
B, T, D, H, Dh, F = 2, 2048, 1024, 16, 64, 4096
BT = B * T
NC_ = 8
TLOC = BT // NC_          # 512 tokens per core
EPS = 1e-6
NSH = 128 * 8 * TLOC      # xhat AllGather shard elements (bf16)


def _patch_tile_tail():
    # This container's walrus rejects the InstISA sem_clear/dma_reset that
    # TileContext emits at kernel tail ("ISA wrong length"). The clears only
    # matter for re-executing a loaded NEFF; skip emitting them and keep the
    # bookkeeping.
    import concourse.bass as bass
    if getattr(bass.Bass, "_acfs_patched", False):
        return
    def _cfs(self, sems):
        if not sems:
            return
        sem_nums = [s.num if hasattr(s, "num") else s for s in sems]
        self._state.prepend_free_semaphores(sem_nums)
        for poison_set in self._tile_sem_poison_stack:
            poison_set.update(sem_nums)
    bass.Bass.clear_and_free_semaphores = _cfs
    bass.Bass._acfs_patched = True


def _legalize_multiwaits(nc):
    # This container's walrus encodes at most ONE semaphore wait per
    # instruction. Tile attaches several. Split: hoist all but the last wait
    # into standalone single-wait EventSemaphore instructions on the same
    # engine, immediately before the original instruction (same block, so
    # per-engine program order is preserved).
    import concourse.mybir as mybir
    wid = 0
    for bb in nc.main_func.blocks:
        il = bb.instructions
        new_list = []
        for inst in il:
            si = getattr(inst, "sync_info", None)
            if si is not None and si.on_wait is not None and len(si.on_wait) > 1:
                waits = list(si.on_wait)
                for w in waits[:-1]:
                    ev = mybir.InstEventSemaphore(name=f"WSPLIT-{wid}", ins=[], outs=[])
                    wid += 1
                    ev.engine = inst.engine
                    ev.sync_info = mybir.SyncInfo(on_wait=[w], on_update=[])
                    new_list.append(ev)
                inst.sync_info = mybir.SyncInfo(on_wait=[waits[-1]],
                                                on_update=list(si.on_update))
            new_list.append(inst)
        il[:] = new_list


def _build(scales):
    import concourse.bass as bass
    import concourse.mybir as mybir
    import concourse.tile as tile
    from concourse.masks import make_identity

    _patch_tile_tail()

    f32 = mybir.dt.float32
    bf16 = mybir.dt.bfloat16
    f8 = mybir.dt.float8e4
    DR = mybir.MatmulPerfMode.DoubleRow
    AF = mybir.ActivationFunctionType
    ALU = mybir.AluOpType
    SQ, SO, SG, SU, SD = (float(scales[k]) for k in ("qkv", "out", "gate", "up", "down"))

    nc = bass.Bass(num_devices=NC_)
    RG = [list(range(NC_))]

    # ---- I/O ----
    x_in = nc.dram_tensor("x_fm", [128, 8, TLOC], f32, kind="ExternalInput")
    wqkv_in = nc.dram_tensor("wqkvT", [128, 8, 384], f8, kind="ExternalInput")
    wo_in = nc.dram_tensor("woT", [128, 8, 1024], bf16, kind="ExternalInput")
    wg_in = nc.dram_tensor("wgT", [4, 128, 8, 1024], f8, kind="ExternalInput")
    wu_in = nc.dram_tensor("wuT", [4, 128, 8, 1024], f8, kind="ExternalInput")
    wd_in = nc.dram_tensor("wdT", [4, 128, 8, 1024], f8, kind="ExternalInput")
    sel_in = nc.dram_tensor("sel16", [16, 8, 128], bf16, kind="ExternalInput")
    out_d = nc.dram_tensor("out", [128, 8, TLOC], f32, kind="ExternalOutput")

    def ap(t, off, dims):
        return bass.AP(tensor=t.tensor, offset=t.offset + off, ap=dims)

    with tile.TileContext(nc) as tc:
        import contextlib
        ctx = contextlib.ExitStack()
        with ctx:
            dram = ctx.enter_context(tc.tile_pool(name="dram", bufs=1, space="DRAM"))
            sing = ctx.enter_context(tc.tile_pool(name="sing", bufs=1))
            # one unified ring of 3x [128,1024] f32 slots (2 banks each) +
            # 2 PV accumulators = 8 PSUM banks
            psMM = ctx.enter_context(tc.tile_pool(name="psMM", bufs=3, space="PSUM"))
            psPV = ctx.enter_context(tc.tile_pool(name="psPV", bufs=2, space="PSUM"))
            meg = ctx.enter_context(tc.tile_pool(name="meg", bufs=4))
            xgp = ctx.enter_context(tc.tile_pool(name="xgp", bufs=3))
            pool = ctx.enter_context(tc.tile_pool(name="pool", bufs=2))

            # ---- DRAM internals ----
            ag_in = dram.tile([NSH], f8, name="ag_in")
            ag_out = dram.tile([NC_ * NSH], f8, name="ag_out", addr_space="Shared")
            # a2a chunk rows: 0:128 = unnormalized attn out (hl0|hl1),
            #                 128:130 = softmax denominators (hl0, hl1)
            a2a_in = dram.tile([NC_, 132, TLOC], bf16, name="a2a_in")
            a2a_out = dram.tile([NC_, 132, TLOC], bf16, name="a2a_out")

            # ---- persistent SBUF ----
            x_fm = sing.tile([128, 8, TLOC], f32, name="x_fm")
            nc.sync.dma_start(out=x_fm[:, 0:4, :], in_=x_in[:, 0:4, :])
            nc.sync.dma_start(out=x_fm[:, 4:8, :], in_=x_in[:, 4:8, :])
            # preload the ln/exp activation table while x streams in
            tldummy = sing.tile([1, 1], f32, name="tldummy")
            nc.vector.memset(tldummy, 1.0)
            nc.scalar.activation(tldummy, tldummy, AF.Ln)
            id_bf = sing.tile([128, 128], bf16, name="id_bf")
            make_identity(nc, id_bf)
            ones_bf = sing.tile([128, 1], bf16, name="ones_bf")
            nc.vector.memset(ones_bf, 1.0)
            ones_row = sing.tile([1, 128], bf16, name="ones_row")
            nc.vector.memset(ones_row, 1.0)
            # head-selector for denominator broadcast: sel16[:, r, :] is a
            # [16, 128] matrix with rows 2r -> out partitions 0:64 and
            # 2r+1 -> out partitions 64:128 (host-prepared)
            sel16 = sing.tile([16, 8, 128], bf16, name="sel16")
            nc.sync.dma_start(out=sel16, in_=sel_in[:, :, :])
            # diagonal-quad causal keep-mask: block t of 4, M[p, t*512+u] = 1
            # iff p <= u - 128*t  (bf16, [128, 2048])
            quadmask = sing.tile([128, 2048], bf16, name="quadmask")
            nc.gpsimd.memset(quadmask, 1.0)
            for t in range(4):
                nc.gpsimd.affine_select(
                    out=quadmask[:, t * 512:(t + 1) * 512],
                    in_=quadmask[:, t * 512:(t + 1) * 512],
                    compare_op=ALU.is_ge, fill=0.0,
                    base=-128 * t, channel_multiplier=-1, pattern=[[1, 512]],
                )
            eps_t = sing.tile([128, 1], f32, name="eps_t")
            nc.vector.memset(eps_t, EPS)

            wqkvT = sing.tile([128, 8, 384], f8, name="wqkvT")
            nc.sync.dma_start(out=wqkvT, in_=wqkv_in[:, :, :])
            xhat = sing.tile([128, 8, TLOC], f8, name="xhat")
            qk_sb = sing.tile([128, 2, BT], bf16, name="qk_sb")
            # token-major v per head: col 64 = ones (softmax denominator trick)
            # j-block stride padded to 80 bytes (fp8) for DoubleRow LDW
            v0 = sing.tile([128, 2, 16, 80], f8, name="v0")
            v1 = sing.tile([128, 2, 16, 80], f8, name="v1")
            nc.vector.memset(v0[:, :, :, 64:65], 1.0)
            nc.vector.memset(v1[:, :, :, 64:65], 1.0)
            x2n = sing.tile([128, 8, TLOC], f8, name="x2n")
            a2a_sb = sing.tile([128, 8, TLOC], bf16, name="a2a_sb")
            a_sb = sing.tile([128, 8, TLOC], f8, name="a_sb")
            rstd1 = sing.tile([1, TLOC], f32, name="rstd1")
            rstd2 = sing.tile([1, TLOC], f32, name="rstd2")

            # bulk weight loads go on the ACT HWDGE ring (nc.scalar) so they
            # never queue ahead of the latency-critical sync-ring DMAs
            wq_tiles = []
            def quarter_weights(qq, eng=None):
                eng = eng or nc.gpsimd
                wg = meg.tile([128, 8, 1024], f8, name=f"wg{qq}", tag="meg")
                eng.dma_start(out=wg, in_=wg_in[qq])
                wu = meg.tile([128, 8, 1024], f8, name=f"wu{qq}", tag="meg")
                eng.dma_start(out=wu, in_=wu_in[qq])
                wd = meg.tile([128, 8, 1024], f8, name=f"wd{qq}", tag="meg")
                eng.dma_start(out=wd, in_=wd_in[qq])
                return wg, wu, wd

            # ============ Phase A1: rmsnorm1 (seq-parallel) ============
            def mm_tile(name):
                t = psMM.tile([128, 1024], f32, name=name, tag="mm")
                return t[:, 0:TLOC]
            psn = mm_tile("psn")
            for dk in range(8):
                sq = pool.tile([128, TLOC], bf16, name="sq", tag="sq", bufs=3)
                nc.vector.tensor_mul(sq, x_fm[:, dk, :], x_fm[:, dk, :])
                nc.tensor.matmul(psn[0:1, :], ones_bf, sq,
                                 start=(dk == 0), stop=(dk == 7))
            # rstd = exp(-0.5 * log(mean + eps))  (keeps the ln/exp table set)
            nc.scalar.activation(rstd1, psn[0:1, :], AF.Ln,
                                 scale=1.0 / D, bias=eps_t[0:1, :])
            nc.scalar.activation(rstd1, rstd1, AF.Exp, scale=-0.5)
            rstd1b = sing.tile([1, TLOC], bf16, name="rstd1b")
            nc.vector.tensor_copy(rstd1b, rstd1)
            # broadcast rstd across partitions via ones-matmul (no DRAM trip)
            ps1b = mm_tile("ps1b")
            nc.tensor.matmul(ps1b, ones_row, rstd1b, start=True, stop=True)
            for dk in range(8):
                nc.vector.tensor_mul(xhat[:, dk, :], x_fm[:, dk, :], ps1b)
            nc.sync.dma_start(out=ap(ag_in, 0, [[4096, 128], [1, 4096]]),
                              in_=xhat.rearrange("p a b -> p (a b)"))

            nc.gpsimd.collective_compute(
                "AllGather", ALU.bypass, replica_groups=RG,
                ins=[ag_in[:].opt()], outs=[ag_out[:].opt()])

            # ---- weight prefetch: clock-delayed so it never contends with
            # the norm1->AllGather critical path ----
            with tc.tile_wait_until(0.055):
                wo = meg.tile([128, 8, 1024], bf16, name="wo", tag="wo")
                nc.scalar.dma_start(out=wo, in_=wo_in[:, :, :])
                wq_tiles.append(quarter_weights(0, nc.scalar))

            # ============ Phase A2: qkv for my 2 heads over ALL tokens ============
            for n in range(8):
                xg = xgp.tile([128, 8, TLOC], f8, name="xg", tag="xg")
                nc.scalar.dma_start(
                    out=xg.rearrange("p a b -> p (a b)"),
                    in_=ap(ag_out, n * NSH, [[4096, 128], [1, 4096]]))
                v_ch = None
                for fb in range(3):
                    ps = mm_tile("psqkv")
                    if DR_QKV:
                        for k2 in range(4):
                            nc.tensor.matmul(
                                ps, wqkvT[:, 2 * k2:2 * k2 + 2, fb * 128:(fb + 1) * 128],
                                xg[:, 2 * k2:2 * k2 + 2, :],
                                start=(k2 == 0), stop=(k2 == 3), perf_mode=DR)
                    else:
                        for dk in range(8):
                            nc.tensor.matmul(
                                ps, wqkvT[:, dk, fb * 128:(fb + 1) * 128],
                                xg[:, dk, :], start=(dk == 0), stop=(dk == 7))
                    if fb < 2:
                        nc.vector.tensor_copy(qk_sb[:, fb, n * 512:(n + 1) * 512], ps)
                    else:
                        v_ch = pool.tile([128, TLOC], bf16, name="v_ch", tag="vch", bufs=2)
                        nc.vector.tensor_copy(v_ch, ps)
                # v -> token-major per head (PE transpose + split-copy)
                tr = psMM.tile([128, 1024], bf16, name="tr", tag="mm")
                for tc4 in range(4):
                    nc.tensor.transpose(tr[:, tc4 * 128:(tc4 + 1) * 128],
                                        v_ch[:, tc4 * 128:(tc4 + 1) * 128], id_bf)
                b, j0 = n // 4, (n % 4) * 4
                for tc4 in range(4):
                    nc.vector.tensor_copy(v0[:, b, j0 + tc4, 0:64],
                                          tr[:, tc4 * 128:tc4 * 128 + 64])
                    nc.vector.tensor_copy(v1[:, b, j0 + tc4, 0:64],
                                          tr[:, tc4 * 128 + 64:tc4 * 128 + 128])

            # ============ Phase B: attention per (b, n) — both heads packed ============
            esc = SQ * SQ * (Dh ** -0.5)
            vs = (v0, v1)
            for b in range(2):
                for n in range(4):
                    pso = [psPV.tile([65, TLOC], f32, name=f"pso{hl}", tag="pv")
                           for hl in range(2)]
                    for pi in range(2 * n + 2):
                        # 2 k-blocks of scores per head into one [128,1024]
                        # f32 PSUM slot; one exp per slot
                        s2s = []
                        for hl in range(2):
                            lo, hi = hl * 64, hl * 64 + 64
                            s2 = psMM.tile([128, 1024], f32, name="s2", tag="mm")
                            for t in range(2):
                                j = 2 * pi + t
                                nc.tensor.matmul(
                                    s2[:, t * 512:(t + 1) * 512],
                                    qk_sb[lo:hi, 1, b * 2048 + j * 128: b * 2048 + (j + 1) * 128],
                                    qk_sb[lo:hi, 0, b * 2048 + n * 512: b * 2048 + (n + 1) * 512],
                                    start=True, stop=True)
                            s2s.append(s2)
                        pts = []
                        for hl in range(2):
                            pt = pool.tile([128, 1024], f8, name="pt", tag="pt", bufs=4)
                            nc.scalar.activation(pt, s2s[hl], AF.Exp, scale=esc)
                            if pi >= 2 * n:
                                moff = (pi - 2 * n) * 1024
                                nc.vector.tensor_mul(pt, pt, quadmask[:, moff:moff + 1024])
                            pts.append(pt)
                        for hl in range(2):
                            nc.tensor.matmul(
                                pso[hl], vs[hl][:, b, 2 * pi:2 * pi + 2, 0:65],
                                pts[hl].rearrange("p (two u) -> p two u", two=2),
                                start=(pi == 0), stop=(pi == 2 * n + 1),
                                perf_mode=DR)
                    for hl in range(2):
                        # unnormalized out + denominator; normalize after A2A
                        o_bf = pool.tile([65, TLOC], bf16, name="o_bf", tag="osb", bufs=2)
                        nc.vector.tensor_copy(o_bf, pso[hl])
                        base = (b * 4 + n) * 132 * 512
                        nc.sync.dma_start(
                            out=ap(a2a_in, base + hl * 64 * 512, [[512, 64], [1, 512]]),
                            in_=o_bf[0:64, :])
                        nc.sync.dma_start(
                            out=ap(a2a_in, base + (128 + hl) * 512, [[512, 1], [1, 512]]),
                            in_=o_bf[64:65, :])

            wq_tiles.append(quarter_weights(1, nc.scalar))

            nc.gpsimd.collective_compute(
                "AllToAll", ALU.bypass, replica_groups=RG,
                ins=[a2a_in[:].opt()], outs=[a2a_out[:].opt()])

            # ============ Phase C: out_proj + residual + rmsnorm2 ============
            den_bf = sing.tile([16, TLOC], bf16, name="den_bf")
            den_sb = sing.tile([16, TLOC], f32, name="den_sb")
            for r in range(NC_):
                nc.sync.dma_start(
                    out=a2a_sb[:, r, :],
                    in_=ap(a2a_out, r * 132 * 512, [[512, 128], [1, 512]]))
                nc.sync.dma_start(
                    out=den_bf[2 * r:2 * r + 2, :],
                    in_=ap(a2a_out, r * 132 * 512 + 128 * 512, [[512, 2], [1, 512]]))
            nc.vector.tensor_copy(den_sb, den_bf)
            nc.vector.reciprocal(den_sb, den_sb)
            nc.vector.tensor_scalar_mul(den_bf, den_sb, SQ)
            for r in range(NC_):
                # rb[0:64] = SQ/den[head 2r], rb[64:128] = SQ/den[head 2r+1]
                psR = mm_tile("psR")
                nc.tensor.matmul(psR, sel16[:, r, :], den_bf,
                                 start=True, stop=True)
                nc.vector.tensor_mul(a2a_sb[:, r, :], a2a_sb[:, r, :], psR)
            for m in range(8):
                ps = mm_tile("psO")
                for r in range(8):
                    nc.tensor.matmul(ps, wo[:, r, m * 128:(m + 1) * 128],
                                     a2a_sb[:, r, :], start=(r == 0), stop=(r == 7))
                nc.vector.scalar_tensor_tensor(
                    out=x_fm[:, m, :], in0=ps, scalar=SO, op0=ALU.mult,
                    op1=ALU.add, in1=x_fm[:, m, :])
            psn2 = mm_tile("psn2")
            for dk in range(8):
                sq2 = pool.tile([128, TLOC], bf16, name="sq2", tag="sq", bufs=3)
                nc.vector.tensor_mul(sq2, x_fm[:, dk, :], x_fm[:, dk, :])
                nc.tensor.matmul(psn2[0:1, :], ones_bf, sq2,
                                 start=(dk == 0), stop=(dk == 7))
            nc.scalar.activation(rstd2, psn2[0:1, :], AF.Ln,
                                 scale=1.0 / D, bias=eps_t[0:1, :])
            nc.scalar.activation(rstd2, rstd2, AF.Exp, scale=-0.5)
            rstd2b = sing.tile([1, TLOC], bf16, name="rstd2b")
            nc.vector.tensor_copy(rstd2b, rstd2)
            ps2b = mm_tile("ps2b")
            nc.tensor.matmul(ps2b, ones_row, rstd2b, start=True, stop=True)
            for dk in range(8):
                nc.vector.tensor_mul(x2n[:, dk, :], x_fm[:, dk, :], ps2b)

            # ============ Phase D: FFN in 4 F-quarters ============
            for qq in range(4):
                if qq > 1:
                    wq_tiles.append(quarter_weights(qq, nc.scalar))
                wg, wu, wd = wq_tiles[qq]
                sgs = {}
                for fb in range(8):
                    psg = mm_tile("psg")
                    if DR_FFN:
                        for k2 in range(4):
                            nc.tensor.matmul(
                                psg, wg[:, 2 * k2:2 * k2 + 2, fb * 128:(fb + 1) * 128],
                                x2n[:, 2 * k2:2 * k2 + 2, :],
                                start=(k2 == 0), stop=(k2 == 3), perf_mode=DR)
                    else:
                        for dk in range(8):
                            nc.tensor.matmul(
                                psg, wg[:, dk, fb * 128:(fb + 1) * 128],
                                x2n[:, dk, :], start=(dk == 0), stop=(dk == 7))
                    sg = pool.tile([128, TLOC], bf16, name="sg", tag="sg", bufs=3)
                    nc.scalar.activation(sg, psg, AF.Silu, scale=SG)
                    sgs[fb] = sg
                    psu = mm_tile("psu")
                    if DR_FFN:
                        for k2 in range(4):
                            nc.tensor.matmul(
                                psu, wu[:, 2 * k2:2 * k2 + 2, fb * 128:(fb + 1) * 128],
                                x2n[:, 2 * k2:2 * k2 + 2, :],
                                start=(k2 == 0), stop=(k2 == 3), perf_mode=DR)
                    else:
                        for dk in range(8):
                            nc.tensor.matmul(
                                psu, wu[:, dk, fb * 128:(fb + 1) * 128],
                                x2n[:, dk, :], start=(dk == 0), stop=(dk == 7))
                    nc.vector.scalar_tensor_tensor(
                        out=a_sb[:, fb, :], in0=psu, scalar=SU, op0=ALU.mult,
                        op1=ALU.mult, in1=sg)
                for m in range(8):
                    psd = mm_tile("psd")
                    if DR_FFN:
                        for k2 in range(4):
                            nc.tensor.matmul(
                                psd, wd[:, 2 * k2:2 * k2 + 2, m * 128:(m + 1) * 128],
                                a_sb[:, 2 * k2:2 * k2 + 2, :],
                                start=(k2 == 0), stop=(k2 == 3), perf_mode=DR)
                    else:
                        for fb in range(8):
                            nc.tensor.matmul(
                                psd, wd[:, fb, m * 128:(m + 1) * 128],
                                a_sb[:, fb, :], start=(fb == 0), stop=(fb == 7))
                    nc.vector.scalar_tensor_tensor(
                        out=x_fm[:, m, :], in0=psd, scalar=SD, op0=ALU.mult,
                        op1=ALU.add, in1=x_fm[:, m, :])

            nc.sync.dma_start(out=out_d[:, :, :], in_=x_fm)
    _legalize_multiwaits(nc)
    return nc


def _tern(w, s):
    return np.clip(np.rint(w / s), -1.0, 1.0).astype(np.float32)


def _prepare(inputs):
    import ml_dtypes
    bf = ml_dtypes.bfloat16
    f8 = ml_dtypes.float8_e4m3
    x = np.asarray(inputs["x"], np.float32).reshape(BT, D)
    qkv_w = np.asarray(inputs["qkv_w"], np.float32)
    out_w = np.asarray(inputs["out_w"], np.float32)
    gate_w = np.asarray(inputs["gate_w"], np.float32)
    up_w = np.asarray(inputs["up_w"], np.float32)
    down_w = np.asarray(inputs["down_w"], np.float32)
    ln1 = np.asarray(inputs["ln1_w"], np.float32)
    ln2 = np.asarray(inputs["ln2_w"], np.float32)

    scales = {
        "qkv": max(np.mean(np.abs(qkv_w), dtype=np.float32), np.float32(1e-5)),
        "out": max(np.mean(np.abs(out_w), dtype=np.float32), np.float32(1e-5)),
        "gate": max(np.mean(np.abs(gate_w), dtype=np.float32), np.float32(1e-5)),
        "up": max(np.mean(np.abs(up_w), dtype=np.float32), np.float32(1e-5)),
        "down": max(np.mean(np.abs(down_w), dtype=np.float32), np.float32(1e-5)),
    }

    # ternary weights, transposed to lhsT tile layouts (bf16; g folds in)
    q3 = _tern(qkv_w, scales["qkv"]) * ln1[None, :]       # [3072, 1024]
    woT = np.ascontiguousarray(
        _tern(out_w, scales["out"]).T.reshape(8, 128, 1024)
        .transpose(1, 0, 2)).astype(bf)                    # [128, 8r, 1024]

    def gu_prep(w):
        a = (_tern(w, scales["gate" if w is gate_w else "up"]) * ln2[None, :]).T
        return np.ascontiguousarray(
            a.reshape(8, 128, 4, 1024).transpose(2, 1, 0, 3)).astype(f8)
    wgT = gu_prep(gate_w)                                  # [4q, 128, 8dk, 1024]
    wuT = gu_prep(up_w)
    wdT = np.ascontiguousarray(
        _tern(down_w, scales["down"]).T.reshape(4, 8, 128, 1024)
        .transpose(0, 2, 1, 3)).astype(f8)                 # [4q, 128, 8fk, 1024]

    sel16 = np.zeros((16, 8, 128), np.float32)
    for r in range(NC_):
        sel16[2 * r, r, 0:64] = 1.0
        sel16[2 * r + 1, r, 64:128] = 1.0
    sel16 = sel16.astype(bf)

    in_maps = []
    for c in range(NC_):
        rows = np.concatenate([
            q3[128 * c:128 * (c + 1)],
            q3[1024 + 128 * c:1024 + 128 * (c + 1)],
            q3[2048 + 128 * c:2048 + 128 * (c + 1)]], axis=0)   # [384, 1024]
        wqkvT = np.ascontiguousarray(
            rows.T.reshape(8, 128, 384).transpose(1, 0, 2)).astype(f8)
        xs = x[TLOC * c:TLOC * (c + 1)]                    # [512, 1024]
        x_fm = np.ascontiguousarray(
            xs.T.reshape(8, 128, TLOC).transpose(1, 0, 2)).astype(np.float32)
        in_maps.append({
            "x_fm": x_fm,
            "wqkvT": wqkvT,
            "woT": woT,
            "wgT": wgT,
            "wuT": wuT,
            "wdT": wdT,
            "sel16": sel16,
        })
    return scales, in_maps


def run(inputs, trace=False):
    from concourse.bass_utils import run_bass_kernel_spmd
    scales, in_maps = _prepare(inputs)
    nc = _build(scales)
    res = run_bass_kernel_spmd(nc, in_maps, list(range(NC_)), trace=trace)
    outs = np.stack([np.asarray(res.results[c]["out"]) for c in range(NC_)])
    # [c, p, dk, t] -> [c, t, dk, p] -> [BT, D]
    y = outs.transpose(0, 3, 2, 1).reshape(BT, D)
    return y.reshape(B, T, D).astype(np.float32), res


def kernel(**inputs):
    out, _ = run(inputs, trace=False)
    return out
